# revision 9
# baseline (speedup 1.0000x reference)
"""Trainium2 Bass kernel for the sparse_attention (channel-attention) module.

Algebraic restructuring: the reference computes, per sample,
    out = BN(Ww @ ((theta @ phi^T)/dim @ g)) + x
with theta/phi/g all 1x1 convs of x / x_h. Folding BN+Ww+Wt into
W2 = diag(inv)·Ww·Wt/dim and collapsing the chain around the Gram matrix
G = x @ xh^T gives

    out = W2 @ G @ Wpg @ xh  +  [rank-1 bias corrections]  + off + x
          (------- device -------)   (---------- host ----------)

where Wpg = Wp^T @ Wg is precomputed. Device work per sample drops from
1812M MACs (6 GEMMs) to 872M MACs (4 GEMMs):
    G   = x^T-major @ xh^T-major   [512,512]   (302M)
    A1T = (W2 @ G)^T               [512,512]   (134M)
    A2T = (A1 @ Wpg)^T             [512,512]   (134M)
    out = A2 @ xh                  [512,1152]  (302M)
All four run in fp8 e4m3 with DoubleRow perf mode (157 TF/s). Every
bias / BN-offset / residual term is mathematically rank-1 and is applied
on the host in fp32 (exact), so the device ships a bias-free chain and
the fp8 output only carries the small attention term (|out_dev| ~ 0.1
vs |x| ~ 4), keeping end-to-end rel err ~7e-4.

Sharding: pure data parallel, 4 samples per core across 8 cores.
Transposed stages (A1T/A2T) make every matmul's stationary operand land
in natural layout with zero on-chip transposes; x^T/xh^T are shipped
n-major from the host (x itself never ships - the residual is host-side).
"""

import numpy as np
import ml_dtypes

import concourse.mybir as mybir
from concourse import bacc
from concourse.tile import TileContext
from concourse import bass_utils

B, DIM, H, W = 32, 512, 48, 24
N = H * W            # 1152
P = 128
CB = DIM // P        # 4 channel blocks
NB = N // P          # 9 n blocks
NBP = 10             # padded n blocks (5 DoubleRow pairs, block 9 = zeros)
NCH = 3
CHW = N // NCH       # 384
NCORES = 8
BL = B // NCORES     # 4 samples per core

_f32 = mybir.dt.float32
_fp8 = mybir.dt.float8e4
_DR = mybir.MatmulPerfMode.DoubleRow
_IDENT = mybir.ActivationFunctionType.Identity

FP8NP = ml_dtypes.float8_e4m3
FP8TGT = 192.0
MARG = np.float32(1.45)

_PROGRAM = None


def _build_program():
    nc = bacc.Bacc("TRN2", target_bir_lowering=False, debug=False)

    # n-major transposed inputs (for G), zero-padded to 10 n-blocks
    xt8 = nc.dram_tensor("xt8", [BL, P, NBP, DIM], _fp8, kind="ExternalInput").ap()
    xht8 = nc.dram_tensor("xht8", [BL, P, NBP, DIM], _fp8, kind="ExternalInput").ap()
    # c-major xh (for the final matmul's moving operand)
    xh8 = nc.dram_tensor("xh8", [BL, P, CB, N], _fp8, kind="ExternalInput").ap()
    # weights: [P, 2(W2T, Wpg), CB, DIM]
    wall = nc.dram_tensor("wall", [P, 2, CB, DIM], _fp8, kind="ExternalInput").ap()
    # per-partition broadcast columns of the 4 eviction scales
    consts = nc.dram_tensor("consts", [P, 16], _f32, kind="ExternalInput").ap()
    # out_dev, chunk-major for per-chunk DMAs
    outd = nc.dram_tensor("outd", [BL, P, NCH, CB, CHW], _fp8,
                          kind="ExternalOutput").ap()

    with TileContext(nc) as tc:
        with tc.tile_pool(name="const", bufs=1) as cpool, \
             tc.tile_pool(name="xin", bufs=4) as xpool, \
             tc.tile_pool(name="xhin", bufs=4) as xhpool, \
             tc.tile_pool(name="mid", bufs=2) as mpool, \
             tc.tile_pool(name="outp", bufs=4) as opool, \
             tc.tile_pool(name="psum", bufs=4, space="PSUM") as psum:

            consts_sb = cpool.tile([P, 16], _f32, tag="consts")
            w_sb = cpool.tile([P, 2, CB, DIM], _fp8, tag="wall")
            w2t_sb = w_sb[:, 0]          # rhs[c, i] = W2[i, c]
            wpg_sb = w_sb[:, 1]          # lhsT[m, j] = Wpg[m, j]

            c_G = consts_sb[:, 0:1]
            c_A1 = consts_sb[:, 1:2]
            c_A2 = consts_sb[:, 2:3]
            c_out = consts_sb[:, 3:4]

            st = [dict() for _ in range(BL)]

            def dma_in_all():
                # ALL input triggers are emitted before any compute op so no
                # prefetch ever queues behind an eviction on its ring
                # (head-of-line blocking cost 7us in an earlier revision).
                # bufs=4 pools mean no buffer-reuse semaphores gate them.
                # Ring budget: sync ~78GB/s but opens earliest -> critical
                # first chunks; gpsimd ~210GB/s -> bulk x/xh transposes;
                # scalar ~78GB/s -> the late-needed c-major xh.
                for s in range(BL):
                    d = st[s]
                    d["xt_sb"] = xpool.tile([P, NBP, DIM], _fp8, tag="xt",
                                            name="xt_sb")
                    d["xht_sb"] = xpool.tile([P, NBP, DIM], _fp8, tag="xht",
                                             name="xht_sb")
                    d["xh_sb"] = xhpool.tile([P, CB, N], _fp8, tag="xh",
                                             name="xh_sb")
                    d["out_sb"] = opool.tile([P, NCH, CB, CHW], _fp8,
                                             tag="osb", name="out_sb")
                # gpsimd is the software-DGE ring: ~210GB/s even on strided
                # patterns. The HWDGE rings (sync/scalar) crawl (~25GB/s) on
                # strided transfers but run ~170GB/s on contiguous ones, so
                # scalar carries the whole-tensor (contiguous) xht1-3, wall
                # and xh8; gpsimd carries everything strided; sync only the
                # tiny consts plus out-chunks later. Splitting xt/xht across
                # two rings roughly doubles early input bandwidth - warmup is
                # input-bound.
                # Whole-tensor transfers are contiguous per partition, so the
                # scalar HWDGE ring moves ALL xht samples + xh8 fast (~170GB/s)
                # while gpsimd streams xt (sample 0 chunked for an early
                # start). wall/consts ride sync, which is otherwise idle early.
                xt0 = st[0]["xt_sb"]
                nc.sync.dma_start(consts_sb, consts)
                nc.sync.dma_start(w_sb, wall)
                for lo, hi in ((0, 2), (2, 6), (6, 10)):
                    nc.gpsimd.dma_start(xt0[:, lo:hi], xt8[0][:, lo:hi])
                for s in range(1, BL):
                    nc.gpsimd.dma_start(st[s]["xt_sb"], xt8[s])
                nc.scalar.dma_start(st[0]["xht_sb"], xht8[0])
                nc.scalar.dma_start(st[1]["xht_sb"], xht8[1])
                nc.scalar.dma_start(st[0]["xh_sb"], xh8[0])
                nc.scalar.dma_start(st[2]["xht_sb"], xht8[2])
                nc.scalar.dma_start(st[1]["xh_sb"], xh8[1])
                nc.scalar.dma_start(st[3]["xht_sb"], xht8[3])
                nc.scalar.dma_start(st[2]["xh_sb"], xh8[2])
                nc.scalar.dma_start(st[3]["xh_sb"], xh8[3])

            def _evict(dst, src, col, h):
                """Split stage evictions across DVE/ACT so both halves of a
                stage drain concurrently (halves inter-stage latency)."""
                if h == 0:
                    nc.vector.tensor_scalar_mul(dst, src, col)
                else:
                    nc.scalar.activation(dst, src, _IDENT, bias=0.0, scale=col)

            def emit_G(s):
                """G[c1, c2] = sum_n xT[n, c1]·xhT[n, c2]; 5 DR pairs over n."""
                d = st[s]
                xt_sb, xht_sb = d["xt_sb"], d["xht_sb"]
                G_sb = mpool.tile([P, CB, DIM], _fp8, tag="G", name="G_sb")
                d["G_sb"] = G_sb
                ps = [psum.tile([P, 2, DIM], _f32, tag="ps2", name="ps2")
                      for _ in range(2)]
                if s == 0:
                    # kb-outer: the first 8 matmuls only need n-blocks 0-3,
                    # which stream in on the early sync-ring chunks.
                    for kb in range(5):
                        for ib in range(4):
                            nc.tensor.matmul(
                                ps[ib // 2][:, ib % 2],
                                xt_sb[:, 2 * kb:2 * kb + 2, ib * P:(ib + 1) * P],
                                xht_sb[:, 2 * kb:2 * kb + 2],
                                start=(kb == 0), stop=(kb == 4), perf_mode=_DR)
                else:
                    for h in range(2):
                        for j in range(2):
                            ib = 2 * h + j
                            for kb in range(5):
                                nc.tensor.matmul(
                                    ps[h][:, j],
                                    xt_sb[:, 2 * kb:2 * kb + 2, ib * P:(ib + 1) * P],
                                    xht_sb[:, 2 * kb:2 * kb + 2],
                                    start=(kb == 0), stop=(kb == 4), perf_mode=_DR)
                for h in range(2):
                    _evict(G_sb[:, 2 * h:2 * h + 2], ps[h], c_G, h)

            def emit_A1T(s):
                """A1T[m, i] = sum_c G[c, m]·W2T[c, i]."""
                d = st[s]
                G_sb = d["G_sb"]
                A1T_sb = mpool.tile([P, CB, DIM], _fp8, tag="A1T", name="A1T_sb")
                d["A1T_sb"] = A1T_sb
                for h in range(2):
                    ps2 = psum.tile([P, 2, DIM], _f32, tag="ps2", name="ps2")
                    for j in range(2):
                        mb = 2 * h + j
                        for k in range(2):
                            nc.tensor.matmul(
                                ps2[:, j],
                                G_sb[:, 2 * k:2 * k + 2, mb * P:(mb + 1) * P],
                                w2t_sb[:, 2 * k:2 * k + 2],
                                start=(k == 0), stop=(k == 1), perf_mode=_DR)
                    _evict(A1T_sb[:, 2 * h:2 * h + 2], ps2, c_A1, h)

            def emit_A2T(s):
                """A2T[j, i] = sum_m Wpg[m, j]·A1T[m, i]."""
                d = st[s]
                A1T_sb = d["A1T_sb"]
                A2T_sb = mpool.tile([P, CB, DIM], _fp8, tag="A2T", name="A2T_sb")
                d["A2T_sb"] = A2T_sb
                for h in range(2):
                    ps2 = psum.tile([P, 2, DIM], _f32, tag="ps2", name="ps2")
                    for j in range(2):
                        jb = 2 * h + j
                        for k in range(2):
                            nc.tensor.matmul(
                                ps2[:, j],
                                wpg_sb[:, 2 * k:2 * k + 2, jb * P:(jb + 1) * P],
                                A1T_sb[:, 2 * k:2 * k + 2],
                                start=(k == 0), stop=(k == 1), perf_mode=_DR)
                    _evict(A2T_sb[:, 2 * h:2 * h + 2], ps2, c_A2, h)

            def emit_OUT(s, chunks, final=False):
                """out[i, nchunk] = sum_j A2T[j, i]·xh[j, nchunk]."""
                d = st[s]
                A2T_sb, xh_sb, out_sb = d["A2T_sb"], d["xh_sb"], d["out_sb"]
                for ch in chunks:
                    last = final and ch == chunks[-1]
                    for h in range(2):
                        ps2 = psum.tile([P, 2, DIM], _f32, tag="ps2", name="ps2")
                        for j in range(2):
                            ib = 2 * h + j
                            for k in range(2):
                                nc.tensor.matmul(
                                    ps2[:, j, :CHW],
                                    A2T_sb[:, 2 * k:2 * k + 2, ib * P:(ib + 1) * P],
                                    xh_sb[:, 2 * k:2 * k + 2,
                                          ch * CHW:(ch + 1) * CHW],
                                    start=(k == 0), stop=(k == 1), perf_mode=_DR)
                            if last:
                                # tail: 4 small evictions alternating engines
                                # so the final one finishes ~0.5us after the
                                # last matmul instead of ~1us
                                _evict(out_sb[:, ch, ib:ib + 1],
                                       ps2[:, j:j + 1, :CHW], c_out, ib % 2)
                        if last:
                            # each half departs the moment its two evictions
                            # complete; the scalar trigger is the last op on
                            # the ACT queue so it delays no eviction, and the
                            # two transfers drain both rings ~1.5us sooner
                            # than one whole-chunk DMA after all four.
                            ring = nc.gpsimd if h == 0 else nc.scalar
                            ring.dma_start(outd[s][:, ch, 2 * h:2 * h + 2],
                                           out_sb[:, ch, 2 * h:2 * h + 2])
                        else:
                            _evict(out_sb[:, ch, 2 * h:2 * h + 2],
                                   ps2[:, :, :CHW], c_out, h)
                    if not last:
                        # whole chunks are contiguous per partition; gpsimd
                        # also handles the strided case fast. sync
                        # (~25-78GB/s) falls behind and then blocks out_sb
                        # reuse. The final sample's chunks all ride gpsimd:
                        # a scalar trigger executes on the ACT queue and
                        # would delay the tail evictions behind it by ~0.6us.
                        ring = nc.gpsimd if (final or (s * NCH + ch) % 2) \
                            else nc.scalar
                        ring.dma_start(outd[s][:, ch], out_sb[:, ch])

            # Software pipeline: fill every inter-stage eviction latency with
            # another sample's independent matmuls.
            # DENSE interleave: the PE clock ramps up only under continuous
            # execution and every idle gap resets it, so each stage's
            # eviction latency must be covered by another sample's matmuls.
            # Early G stages are spaced to match the input rings' delivery.
            dma_in_all()
            # Dummy matmuls on the (tiny, early-arriving) consts tile keep the
            # PE busy from ~11us until sample 0's data lands, pre-ramping the
            # clock. Their psum tile is never read; a later start=True reuse
            # wipes it.
            ps_warm = psum.tile([P, 2, DIM], _f32, tag="ps2", name="ps_warm")
            for _ in range(16):
                nc.tensor.matmul(ps_warm[0:2, 0, 0:16], consts_sb[:, 0:2],
                                 consts_sb, start=True, stop=True)
            emit_G(0)
            emit_G(1)
            emit_A1T(0)
            emit_G(2)
            emit_A2T(0)
            emit_A1T(1)
            emit_OUT(0, [0, 1, 2])
            emit_A2T(1)
            emit_G(3)
            emit_OUT(1, [0])
            emit_A1T(2)
            emit_OUT(1, [1, 2])
            emit_A2T(2)
            emit_A1T(3)
            emit_OUT(2, [0, 1])
            emit_A2T(3)
            emit_OUT(2, [2])
            emit_OUT(3, [0, 1, 2], final=True)

    nc.finalize()
    return nc


def _get_program():
    global _PROGRAM
    if _PROGRAM is None:
        _PROGRAM = _build_program()
    return _PROGRAM


def _q8(a, scale):
    return np.asarray(a.astype(np.float32) * np.float32(scale)).astype(FP8NP)


def _prep(x, x_h, Wg, bg, Wt, bt, Wp, bp, Ww, bw, gamma, beta, run_mean,
          run_var):
    f32 = np.float32
    x = x.reshape(B, DIM, N).astype(f32)
    xh = x_h.reshape(B, DIM, N).astype(f32)
    Wg, bg = Wg.astype(f32), bg.astype(f32)
    Wt, bt = Wt.astype(f32), bt.astype(f32)
    Wp, bp = Wp.astype(f32), bp.astype(f32)
    Ww, bw = Ww.astype(f32), bw.astype(f32)
    inv = (gamma.astype(f32) / np.sqrt(run_var.astype(f32) + f32(1e-5)))
    off = (bw - run_mean.astype(f32)) * inv + beta.astype(f32)

    W2 = (inv[:, None] * Ww) @ Wt / f32(DIM)       # [i, c]
    b2 = (inv[:, None] * Ww) @ bt / f32(DIM)       # [i]
    Wpg = Wp.T @ Wg                                 # [m, j]... [c2, c]
    Wgb = Wg.T @ bp                                 # [c]
    Wpbg = Wp.T @ bg                                # [c2]
    pbg = f32(bp @ bg)

    # ---- scales (inputs/weights exact; intermediates: sample-0 + margin) ----
    G0 = x[0] @ xh[0].T
    A10 = W2 @ G0
    A20 = A10 @ Wpg
    o0 = A20 @ xh[0]

    def s_of(a, m=MARG):
        return f32(FP8TGT / (np.abs(a).max() * m))

    s_x = s_of(x, f32(1.0))
    s_xh = s_of(xh, f32(1.0))
    s_W2 = s_of(W2, f32(1.0))
    s_Wpg = s_of(Wpg, f32(1.0))
    s_G, s_A1, s_A2, s_o = s_of(G0), s_of(A10), s_of(A20), s_of(o0)

    # ---- host-side rank-1 corrections (exact fp32) ----
    xs = x.sum(2)                                   # [B, c1]
    xhs = xh.sum(2)                                 # [B, c2]
    u1 = xs @ W2.T                                  # [B, i]
    vpx = xhs @ Wp.T                                # [B, j] = Wp @ xhs
    qv = vpx @ Wg + f32(N) * Wgb[None]              # [B, c]
    r1 = np.einsum('c,bcn->bn', Wgb, xh)            # [B, n]
    r2 = np.einsum('bc,bcn->bn', qv, xh)            # [B, n]
    v2p = vpx + f32(N) * bp[None]                   # [B, j]
    z1 = np.einsum('c,bcn->bn', Wpbg, xh)           # [B, n]
    z2 = np.einsum('bn,bcn->bc', z1, x)             # [B, c1]
    cmain = z2 @ W2.T                               # [B, i]
    c = cmain + pbg * u1 + (v2p @ bg)[:, None] * b2[None]

    # corr = x + (c + off) + u1 (x) r1 + b2 (x) r2  (everything but out_dev)
    corr = x + (c + off[None])[:, :, None]
    corr += u1[:, :, None] * r1[:, None, :]
    corr += b2[None, :, None] * r2[:, None, :]

    # ---- device tensors ----
    wstack = np.empty((P, 2, CB, DIM), dtype=FP8NP)
    wstack[:, 0] = _q8(W2.T.reshape(CB, P, DIM), s_W2).transpose(1, 0, 2)
    wstack[:, 1] = _q8(Wpg.reshape(CB, P, DIM), s_Wpg).transpose(1, 0, 2)

    consts = np.zeros((P, 16), dtype=f32)
    consts[:, 0] = s_G / (s_x * s_xh)
    consts[:, 1] = s_A1 / (s_W2 * s_G)
    consts[:, 2] = s_A2 / (s_A1 * s_Wpg)
    consts[:, 3] = s_o / (s_A2 * s_xh)

    def tmajor(a, s):
        # [BL, c, n] -> [BL, P, NBP, DIM] n-major fp8, zero-padded block 9
        q = _q8(a, s)                                # [BL, 512, 1152]
        out = np.zeros((a.shape[0], P, NBP, DIM), dtype=FP8NP)
        out[:, :, :NB] = q.transpose(0, 2, 1).reshape(
            a.shape[0], NB, P, DIM).transpose(0, 2, 1, 3)
        return out

    def cmajor(a, s):
        # [BL, c, n] -> [BL, P, CB, N] fp8
        return np.ascontiguousarray(
            _q8(a, s).reshape(a.shape[0], CB, P, N).transpose(0, 2, 1, 3))

    in_maps = []
    for k in range(NCORES):
        sl = slice(k * BL, (k + 1) * BL)
        in_maps.append(dict(
            xt8=tmajor(x[sl], s_x),
            xht8=tmajor(xh[sl], s_xh),
            xh8=cmajor(xh[sl], s_xh),
            wall=wstack,
            consts=consts,
        ))
    return in_maps, corr, f32(1.0 / s_o)


def run(inputs, trace=False, tmpdir=None):
    nc = _get_program()
    in_maps, corr, o_deq = _prep(**inputs)
    res = bass_utils.run_bass_kernel_spmd(
        nc, in_maps, core_ids=list(range(NCORES)), trace=trace, tmpdir=tmpdir)
    outs = [r["outd"] for r in res.results]     # each [BL, P, NCH, CB, CHW]
    od = np.concatenate(outs, axis=0).astype(np.float32) * o_deq
    # [B, P, NCH, CB, CHW] -> out_dev[b, ib*128+p, ch*384+w]
    od = od.transpose(0, 3, 1, 2, 4).reshape(B, DIM, N)
    out = (od + corr).reshape(B, DIM, H, W)
    return out.astype(np.float32), res


def kernel(**inputs) -> np.ndarray:
    out, _ = run(inputs)
    return out
